# revision 1
# baseline (speedup 1.0000x reference)
"""CategoryCrossAttention Trainium2 kernel — 8 NeuronCores, data-parallel over B.

Reference computation (B=32, T=2048, D=1024, C=512, H=1024):
    xn  = LN(x) ; cn = LN(cat_emb)
    q   = cn @ Wq.T                       [B,H]
    k,v = xn @ Wk.T, xn @ Wv.T            [B,T,H]   <- 2 x 137 GFLOP GEMMs
    wei = softmax(q.k / sqrt(H))          [B,T]
    out = sum_t wei*v ; proj = out @ Wp.T ; y = x + proj[:,None,:]

Algebraic rewrite (exact): contract the weights with the small side first.
    logits[b,t] = kq_b . xn[b,t]   with kq_b = (q_b/sqrt(H)) @ Wk      [D]
                = rstd[t]*(x[t].kqg - mu[t]*S) + kq.beta,  kqg = g*kq
    xw[b,d] = sum_t w2[t]*x[t,d] - S2,  w2 = wei*rstd, S2 = sum_t w2[t]*mu[t]
    out_b   = (g*xw) @ Wv.T + (beta @ Wv.T)   ;  proj_b = out_b @ Wp.T
This removes all O(B*T*D*H) GEMMs; the kernel is HBM-bound:
read x once into SBUF, two on-chip passes, write y once.

Sharding: batch-parallel, 4 batches per core, no collectives.
"""
import numpy as np
from contextlib import ExitStack

import concourse.bass as bass
import concourse.tile as tile
from concourse import mybir, masks
from concourse.bass_utils import run_bass_kernel_spmd
from concourse.vector_clock import ScopedClock

B, T, D, C, H = 32, 2048, 1024, 512, 1024
NCORES = 8
BL = B // NCORES          # 4 batches per core
NT = T // 128             # 16 row tiles per batch
ND = D // 128
NH = H // 128
NCC = C // 128
EPS = 1e-5

F32 = mybir.dt.float32
F32R = mybir.dt.float32r
BF16 = mybir.dt.bfloat16
AX = mybir.AxisListType
OP = mybir.AluOpType
AF = mybir.ActivationFunctionType


# ---------------------------------------------------------------------------
# Walrus in this container encodes at most ONE sem wait per instruction.
# Two workarounds: (1) the Tile kernel-tail drain aggregates many waits ->
# replace with single-wait NOPs; (2) post-pass hoists extra waits from any
# instruction onto single-wait NOPs inserted before it (same engine).
# ---------------------------------------------------------------------------
class PatchedTileContext(tile.TileContext):
    def _drain_and_barrier(self, tick_clock, wait_clock):
        probe = self.nc.sync.nop()
        wait_clock.add_sem_waits(
            probe.ins, ScopedClock({None: tick_clock.global_clock})
        )
        si = probe.ins.sync_info
        waits = list(si.on_wait) if si and si.on_wait else []
        if len(waits) > 1:
            probe.ins.sync_info = mybir.SyncInfo(
                on_wait=waits[:1], on_update=list(si.on_update or [])
            )
            for w in waits[1:]:
                n2 = self.nc.sync.nop()
                n2.ins.sync_info = mybir.SyncInfo(on_wait=[w], on_update=[])
        self.nc.sync.drain()
        self.nc.all_engine_barrier()
        popped = self.nc._tile_sem_poison_stack.pop()
        assert popped is self._sem_poison
        self.nc.clear_and_free_semaphores(list(self.sems.allocated().values()))
        self.nc.all_engine_barrier()


_SEQ = [0]


def split_multi_waits(nc):
    for f in nc.m.functions:
        for bb in f.blocks:
            insts = bb.instructions
            need = False
            for i in insts:
                si = i.sync_info
                if si is not None and si.on_wait and len(si.on_wait) > 1:
                    need = True
                    break
            if not need:
                continue
            new = []
            for inst in insts:
                si = inst.sync_info
                waits = list(si.on_wait) if si is not None and si.on_wait else []
                if len(waits) > 1:
                    for w in waits[:-1]:
                        _SEQ[0] += 1
                        n = mybir.InstNoOp(
                            name=f"waitsplit_{_SEQ[0]}", engine=inst.engine
                        )
                        n.sync_info = mybir.SyncInfo(on_wait=[w], on_update=[])
                        new.append(n)
                    inst.sync_info = mybir.SyncInfo(
                        on_wait=[waits[-1]], on_update=list(si.on_update or [])
                    )
                new.append(inst)
            bb.instructions = new


# ---------------------------------------------------------------------------
# Kernel body
# ---------------------------------------------------------------------------
def _mm(nc, out, lhsT, rhs, start, stop):
    nc.tensor.matmul(out, lhsT, rhs, start=start, stop=stop)


def build_body(ctx, tc, ext):
    nc = tc.nc
    x_ext = ext["x"]
    out_ext = ext["out"]

    # --- persistent pools -------------------------------------------------
    const_p = ctx.enter_context(tc.tile_pool(name="const", bufs=1))
    wvT_p = ctx.enter_context(tc.tile_pool(name="wvT", bufs=1))
    wpT_p = ctx.enter_context(tc.tile_pool(name="wpT", bufs=1))
    kqg_p = ctx.enter_context(tc.tile_pool(name="kqg", bufs=1))
    perb_p = ctx.enter_context(tc.tile_pool(name="perb", bufs=2))
    rows_p = ctx.enter_context(tc.tile_pool(name="rows", bufs=2))
    junk_p = ctx.enter_context(tc.tile_pool(name="junk", bufs=1))
    pjbc_p = ctx.enter_context(tc.tile_pool(name="pjbc", bufs=1))
    # PSUM: rowp [<=4,1024] (2 banks) x2 ; tp [<=128,512] (1 bank) x2 ;
    # big [128,1024] (2 banks) x1  => 8 banks total
    ps_row = ctx.enter_context(tc.tile_pool(name="psrow", bufs=2, space="PSUM"))
    ps_tp = ctx.enter_context(tc.tile_pool(name="pstp", bufs=2, space="PSUM"))
    ps_big = ctx.enter_context(tc.tile_pool(name="psbig", bufs=1, space="PSUM"))

    # --- constants --------------------------------------------------------
    ident = const_p.tile([128, 128], F32)
    masks.make_identity(nc, ident[:])
    ident_bf = const_p.tile([128, 128], BF16)
    masks.make_identity(nc, ident_bf[:])
    ones_row = const_p.tile([1, 128], F32)
    nc.vector.memset(ones_row[:], 1.0)
    eps128 = const_p.tile([128, 1], F32)
    nc.vector.memset(eps128[:], EPS)
    eps4 = const_p.tile([BL, 1], F32)
    nc.vector.memset(eps4[:], EPS)
    # aligned e_b selector columns: ecol[:, b, 0] = identity column b
    ecol = const_p.tile([BL, BL, 4], F32)
    nc.vector.memset(ecol[:], 0.0)
    nc.vector.tensor_copy(ecol[:, :, 0], ident[:BL, :BL])
    g_row = const_p.tile([1, D], F32)
    nc.scalar.dma_start(g_row[:], ext["ln_x_g"][:, :])
    b_row = const_p.tile([1, D], F32)
    nc.scalar.dma_start(b_row[:], ext["ln_x_b"][:, :])

    def bcast_col(colap, b, tag="bc1"):
        """[BL,1] column value at partition b -> [128,1] tile (all partitions)."""
        # extract to partition 0: e_b.T @ col -> [1,1]
        ex_ps = ps_tp.tile([1, 1], F32, tag="tp")
        _mm(nc, ex_ps[:], ecol[:, b, 0:1], colap, start=True, stop=True)
        ex = perb_p.tile([1, 1], F32, tag=f"x{tag}", name=f"ex_{tag}")
        nc.scalar.copy(ex[:], ex_ps[:])
        ps = ps_tp.tile([128, 1], F32, tag="tp")
        _mm(nc, ps[:], ones_row[:], ex[:], start=True, stop=True)
        sb = perb_p.tile([128, 1], F32, tag=tag, name=f"bc_{tag}")
        nc.scalar.copy(sb[:], ps[:])
        return sb

    # --- phase 0a: Wq -> WqT; cat LN -> q --------------------------------
    with ExitStack() as s0:
        wq_p = s0.enter_context(tc.tile_pool(name="wqnat", bufs=1))
        wqT_p = s0.enter_context(tc.tile_pool(name="wqT", bufs=1))
        st0_p = s0.enter_context(tc.tile_pool(name="st0", bufs=1))
        wq_nat = []
        for hc in range(NH):
            wt = wq_p.tile([128, C], F32, tag=f"wq{hc}", name=f"wqnat{hc}")
            nc.scalar.dma_start(wt[:], ext["Wq"][hc * 128 : (hc + 1) * 128, :])
            wq_nat.append(wt)
        wqT = []
        for cc in range(NCC):
            wqT.append(wqT_p.tile([128, H], F32, tag=f"wqT{cc}", name=f"wqT{cc}"))
        for cc in range(NCC):
            for g4 in range(0, NH, 4):
                tp = ps_tp.tile([128, 512], F32, tag="tp")
                for j in range(4):
                    nc.tensor.transpose(
                        tp[:, j * 128 : (j + 1) * 128],
                        wq_nat[g4 + j][:, cc * 128 : (cc + 1) * 128],
                        ident[:],
                    )
                nc.scalar.copy(wqT[cc][:, g4 * 128 : (g4 + 4) * 128], tp[:])

        # cat LN
        cat_sb = st0_p.tile([BL, C], F32, tag="cat")
        nc.scalar.dma_start(cat_sb[:], ext["cat_emb"][:, :])
        st6c = st0_p.tile([BL, 6], F32, tag="st6c")
        nc.vector.bn_stats(st6c[:], cat_sb[:])
        stc = st0_p.tile([BL, 2], F32, tag="stc")
        nc.vector.bn_aggr(stc[:], st6c[:])
        sdc = st0_p.tile([BL, 1], F32, tag="sdc")
        nc.scalar.activation(sdc[:], stc[:, 1:2], AF.Sqrt, bias=eps4[:], scale=1.0)
        rstdc = st0_p.tile([BL, 1], F32, tag="rstdc")
        nc.vector.reciprocal(rstdc[:], sdc[:])
        nbc = st0_p.tile([BL, 1], F32, tag="nbc")
        nc.vector.tensor_tensor(out=nbc[:], in0=stc[:, 0:1], in1=rstdc[:], op=OP.mult)
        nc.scalar.mul(nbc[:], nbc[:], -1.0)
        zc = st0_p.tile([BL, C], F32, tag="zc")
        nc.scalar.activation(
            zc[:], cat_sb[:], AF.Identity, bias=nbc[:], scale=rstdc[:]
        )
        # cn = zc * g_c + b_c  (broadcast LN params to BL partitions)
        gc_ps = ps_big.tile([BL, C], F32, tag="big")
        gcr = st0_p.tile([1, C], F32, tag="gcr")
        nc.scalar.dma_start(gcr[:], ext["ln_c_g"][:, :])
        bcr = st0_p.tile([1, C], F32, tag="bcr")
        nc.scalar.dma_start(bcr[:], ext["ln_c_b"][:, :])
        _mm(nc, gc_ps[:], ones_row[:, :BL], gcr[:], start=True, stop=True)
        gc4 = st0_p.tile([BL, C], F32, tag="gc4")
        nc.scalar.copy(gc4[:], gc_ps[:])
        bc_ps = ps_big.tile([BL, C], F32, tag="big")
        _mm(nc, bc_ps[:], ones_row[:, :BL], bcr[:], start=True, stop=True)
        bc4 = st0_p.tile([BL, C], F32, tag="bc4")
        nc.scalar.copy(bc4[:], bc_ps[:])
        cn = st0_p.tile([BL, C], F32, tag="cn")
        nc.vector.tensor_tensor(out=cn[:], in0=zc[:], in1=gc4[:], op=OP.mult)
        nc.vector.tensor_tensor(out=cn[:], in0=cn[:], in1=bc4[:], op=OP.add)
        # cnT columns [128c x BL] x NCC
        cnT_ps = ps_tp.tile([128, NCC * BL], F32, tag="tp")
        for cc in range(NCC):
            nc.tensor.transpose(
                cnT_ps[:, cc * BL : (cc + 1) * BL],
                cn[:, cc * 128 : (cc + 1) * 128],
                ident[:BL, :BL],
            )
        cnT = st0_p.tile([128, NCC * BL], F32, tag="cnT")
        nc.scalar.copy(cnT[:], cnT_ps[:])
        # q = cn @ Wq.T  -> [BL, H] ; scale by H^-0.5
        q_ps = ps_row.tile([BL, H], F32, tag="rowp")
        for cc in range(NCC):
            for hf in range(2):
                _mm(
                    nc,
                    q_ps[:, hf * 512 : (hf + 1) * 512],
                    cnT[:, cc * BL : (cc + 1) * BL],
                    wqT[cc][:, hf * 512 : (hf + 1) * 512],
                    start=(cc == 0),
                    stop=(cc == NCC - 1),
                )
        q_sb = st0_p.tile([BL, H], F32, tag="qsb")
        nc.scalar.mul(q_sb[:], q_ps[:], float(H) ** -0.5)
        # qT columns [128h x BL] x NH
        qT_ps = ps_tp.tile([128, NH * BL], F32, tag="tp")
        for hc in range(NH):
            nc.tensor.transpose(
                qT_ps[:, hc * BL : (hc + 1) * BL],
                q_sb[:, hc * 128 : (hc + 1) * 128],
                ident[:BL, :BL],
            )
        qT = rows_p.tile([128, NH * BL], F32, tag="qT")
        nc.scalar.copy(qT[:], qT_ps[:])

    # --- phase 0b: Wk -> kq, kqg, S, cbeta -------------------------------
    kqg_bc = []
    for b in range(BL):
        kqg_bc.append(
            kqg_p.tile([128, 1, D], BF16, tag=f"kqgbc{b}", name=f"kqgbc{b}")
        )
    with ExitStack() as s1:
        wk_p = s1.enter_context(tc.tile_pool(name="wknat", bufs=1))
        st1_p = s1.enter_context(tc.tile_pool(name="st1", bufs=1))
        wk_nat = []
        for hc in range(NH):
            wt = wk_p.tile([128, D], F32, tag=f"wk{hc}", name=f"wknat{hc}")
            nc.scalar.dma_start(wt[:], ext["Wk"][hc * 128 : (hc + 1) * 128, :])
            wk_nat.append(wt)
        kq_ps = ps_row.tile([BL, D], F32, tag="rowp")
        for hc in range(NH):
            for df in range(2):
                _mm(
                    nc,
                    kq_ps[:, df * 512 : (df + 1) * 512],
                    qT[:, hc * BL : (hc + 1) * BL],
                    wk_nat[hc][:, df * 512 : (df + 1) * 512],
                    start=(hc == 0),
                    stop=(hc == NH - 1),
                )
        kq_sb = st1_p.tile([BL, D], F32, tag="kqsb")
        nc.scalar.copy(kq_sb[:], kq_ps[:])
        # g_x, b_x broadcast to BL partitions
        gx_ps = ps_big.tile([BL, D], F32, tag="big")
        for df in range(2):
            _mm(
                nc,
                gx_ps[:, df * 512 : (df + 1) * 512],
                ones_row[:, :BL],
                g_row[:, df * 512 : (df + 1) * 512],
                start=True,
                stop=True,
            )
        gx4 = st1_p.tile([BL, D], F32, tag="gx4")
        nc.scalar.copy(gx4[:], gx_ps[:])
        bx_ps = ps_big.tile([BL, D], F32, tag="big")
        for df in range(2):
            _mm(
                nc,
                bx_ps[:, df * 512 : (df + 1) * 512],
                ones_row[:, :BL],
                b_row[:, df * 512 : (df + 1) * 512],
                start=True,
                stop=True,
            )
        bx4 = st1_p.tile([BL, D], F32, tag="bx4")
        nc.scalar.copy(bx4[:], bx_ps[:])
        kqg4 = st1_p.tile([BL, D], F32, tag="kqg4")
        nc.vector.tensor_tensor(out=kqg4[:], in0=kq_sb[:], in1=gx4[:], op=OP.mult)
        S4 = st1_p.tile([BL, 1], F32, tag="S4")
        nc.vector.tensor_reduce(S4[:], kqg4[:], axis=AX.X, op=OP.add)
        jk = st1_p.tile([BL, D], F32, tag="junk4")
        nc.vector.tensor_tensor(out=jk[:], in0=kq_sb[:], in1=bx4[:], op=OP.mult)
        cb4 = st1_p.tile([BL, 1], F32, tag="cb4")
        nc.vector.tensor_reduce(cb4[:], jk[:], axis=AX.X, op=OP.add)
        # broadcast kqg rows to full [128, D] tiles (per batch)
        for b in range(BL):
            kr_ps = ps_row.tile([1, D], F32, tag="rowp")
            for df in range(2):
                _mm(
                    nc,
                    kr_ps[:, df * 512 : (df + 1) * 512],
                    ecol[:, b, 0:1],
                    kqg4[:, df * 512 : (df + 1) * 512],
                    start=True,
                    stop=True,
                )
            krow = st1_p.tile([1, D], F32, tag="krow", name=f"krow{b}")
            nc.scalar.copy(krow[:], kr_ps[:])
            kb_ps = ps_big.tile([128, D], F32, tag="big")
            for df in range(2):
                _mm(
                    nc,
                    kb_ps[:, df * 512 : (df + 1) * 512],
                    ones_row[:],
                    krow[:, df * 512 : (df + 1) * 512],
                    start=True,
                    stop=True,
                )
            nc.scalar.copy(kqg_bc[b][:, 0, :], kb_ps[:])
        S_bc = [bcast_col(S4[:], b, tag=f"Sbc{b}") for b in range(BL)]
        cb_bc = [bcast_col(cb4[:], b, tag=f"cbbc{b}") for b in range(BL)]

    # --- phase 0c: Wv -> wvT(bf16) + cvb ; Wp -> wpT(bf16) + cpv ---------
    # Emitted lazily (after batch 0 pass 1) so its ACT/PE work doesn't block
    # the first batch's casts in the per-engine instruction FIFOs.
    wvT = [wvT_p.tile([128, H], BF16, tag=f"wvT{dc}", name=f"wvT{dc}") for dc in range(ND)]
    wpT = [wpT_p.tile([128, D], BF16, tag=f"wpT{hc}", name=f"wpT{hc}") for hc in range(NH)]
    cvb_bf = const_p.tile([128, NH], BF16)
    cpv_row = const_p.tile([1, D], F32)

    def phase0c_chunks():
        with ExitStack() as s2:
            wv_p = s2.enter_context(tc.tile_pool(name="wvnat", bufs=4))
            st2_p = s2.enter_context(tc.tile_pool(name="st2", bufs=1))
            # beta broadcast [128, D] for cvb
            bb_ps = ps_big.tile([128, D], F32, tag="big")
            for df in range(2):
                _mm(
                    nc,
                    bb_ps[:, df * 512 : (df + 1) * 512],
                    ones_row[:],
                    b_row[:, df * 512 : (df + 1) * 512],
                    start=True,
                    stop=True,
                )
            bb128 = st2_p.tile([128, D], F32, tag="bb128")
            nc.scalar.copy(bb128[:], bb_ps[:])
            cvb = st2_p.tile([128, NH], F32, tag="cvb")
            for hc in range(NH):
                wt = wv_p.tile([128, D], F32, tag="wvnat", name=f"wvnat{hc}")
                nc.scalar.dma_start(wt[:], ext["Wv"][hc * 128 : (hc + 1) * 128, :])
                jv = st2_p.tile([128, D], F32, tag="jv", bufs=1, name=f"jv{hc}")
                nc.vector.tensor_tensor(
                    out=jv[:], in0=wt[:], in1=bb128[:], op=OP.mult
                )
                nc.vector.tensor_reduce(
                    cvb[:, hc : hc + 1], jv[:], axis=AX.X, op=OP.add
                )
                # wvT column blocks from this row chunk
                for dg in range(0, ND, 4):
                    tp = ps_tp.tile([128, 512], F32, tag="tp")
                    for j in range(4):
                        nc.tensor.transpose(
                            tp[:, j * 128 : (j + 1) * 128],
                            wt[:, (dg + j) * 128 : (dg + j + 1) * 128],
                            ident[:],
                        )
                    for j in range(4):
                        nc.scalar.copy(
                            wvT[dg + j][:, hc * 128 : (hc + 1) * 128],
                            tp[:, j * 128 : (j + 1) * 128],
                        )
                yield
            nc.vector.tensor_copy(cvb_bf[:], cvb[:])
        with ExitStack() as s3:
            wp_p = s3.enter_context(tc.tile_pool(name="wpnat", bufs=4))
            for dc in range(ND):
                wt = wp_p.tile([128, H], F32, tag="wpnat", name=f"wpnat{dc}")
                nc.scalar.dma_start(wt[:], ext["Wp"][dc * 128 : (dc + 1) * 128, :])
                for hg in range(0, NH, 4):
                    tp = ps_tp.tile([128, 512], F32, tag="tp")
                    for j in range(4):
                        nc.tensor.transpose(
                            tp[:, j * 128 : (j + 1) * 128],
                            wt[:, (hg + j) * 128 : (hg + j + 1) * 128],
                            ident[:],
                        )
                    for j in range(4):
                        nc.scalar.copy(
                            wpT[hg + j][:, dc * 128 : (dc + 1) * 128],
                            tp[:, j * 128 : (j + 1) * 128],
                        )
                yield
            # cpv = cvb @ Wp.T  (row [1, D])
            cpv_ps = ps_row.tile([1, D], F32, tag="rowp")
            for hc in range(NH):
                for df in range(2):
                    nc.tensor.matmul(
                        cpv_ps[:, df * 512 : (df + 1) * 512],
                        cvb_bf[:, hc : hc + 1],
                        wpT[hc][:, df * 512 : (df + 1) * 512],
                        start=(hc == 0),
                        stop=(hc == NH - 1),
                    )
            nc.scalar.copy(cpv_row[:], cpv_ps[:])

    # --- main loop over batches ------------------------------------------
    # x lives in SBUF as bf16 (used by: logit dot, xw matmul, y-add).
    # f32 arrival tiles are short-lived staging (bn_stats reads them).
    p0c = phase0c_chunks()

    def drive_p0c(n):
        for _ in range(n):
            try:
                next(p0c)
            except StopIteration:
                break

    drive_p0c(1000)  # emit all of phase 0c upfront (v2 schedule)
    xbf_p = ctx.enter_context(tc.tile_pool(name="xbf", bufs=26))
    xf_p = ctx.enter_context(tc.tile_pool(name="xf", bufs=6))
    y_p = ctx.enter_context(tc.tile_pool(name="y", bufs=5))
    for b in range(BL):
        x_tiles = []
        stats_b = perb_p.tile([128, NT, 2], F32, tag="stats")
        dots_b = perb_p.tile([128, NT], F32, tag="dots")
        # pass 1: stream tiles, LN stats + logit dot
        for ti in range(NT):
            xf = xf_p.tile([128, D], F32, tag="xf")
            nc.sync.dma_start(xf[:], x_ext[b, ti * 128 : (ti + 1) * 128, :])
            st6 = perb_p.tile([128, 2, 6], F32, tag="st6")
            nc.vector.bn_stats(st6[:, 0, :], xf[:, 0:512])
            nc.vector.bn_stats(st6[:, 1, :], xf[:, 512:1024])
            nc.vector.bn_aggr(stats_b[:, ti, :], st6[:])
            xt = xbf_p.tile([128, D], BF16, tag="x")
            nc.scalar.copy(xt[:], xf[:])
            x_tiles.append(xt)
            pj = junk_p.tile([128, D], BF16, tag="prod", bufs=3)
            nc.vector.tensor_tensor(
                out=pj[:], in0=xt[:], in1=kqg_bc[b][:, 0, :], op=OP.mult
            )
            pj2 = junk_p.tile([128, D], BF16, tag="prod2", bufs=3)
            nc.scalar.activation(
                pj2[:], pj[:], AF.Identity, bias=0.0, scale=1.0,
                accum_out=dots_b[:, ti : ti + 1],
            )
        mu_v = stats_b[:, :, 0]
        var_v = stats_b[:, :, 1]
        # logits = (dots - mu*S)*rstd + cbeta
        rstd = perb_p.tile([128, NT], F32, tag="rstd")
        nc.scalar.activation(rstd[:], var_v, AF.Sqrt, bias=eps128[:], scale=1.0)
        nc.vector.reciprocal(rstd[:], rstd[:])
        lg = perb_p.tile([128, NT], F32, tag="lg")
        nc.vector.tensor_scalar(lg[:], mu_v, S_bc[b][:], None, OP.mult)
        nc.vector.tensor_tensor(out=lg[:], in0=dots_b[:], in1=lg[:], op=OP.subtract)
        nc.vector.tensor_tensor(out=lg[:], in0=lg[:], in1=rstd[:], op=OP.mult)
        nc.vector.tensor_scalar(lg[:], lg[:], cb_bc[b][:], None, OP.add)
        # softmax over all T (cross-partition via PE transpose)
        m1 = perb_p.tile([128, 1], F32, tag="m1")
        nc.vector.tensor_reduce(m1[:], lg[:], axis=AX.X, op=OP.max)
        m1t_ps = ps_tp.tile([1, 128], F32, tag="tp")
        nc.tensor.transpose(m1t_ps[:], m1[:], ident[:])
        m1r = perb_p.tile([1, 128], F32, tag="m1r")
        nc.scalar.copy(m1r[:], m1t_ps[:])
        m2 = perb_p.tile([1, 1], F32, tag="m2")
        nc.vector.tensor_reduce(m2[:], m1r[:], axis=AX.X, op=OP.max)
        nc.scalar.mul(m2[:], m2[:], -1.0)
        nmx_ps = ps_tp.tile([128, 1], F32, tag="tp")
        _mm(nc, nmx_ps[:], ones_row[:], m2[:], start=True, stop=True)
        nmx = perb_p.tile([128, 1], F32, tag="nmx")
        nc.scalar.copy(nmx[:], nmx_ps[:])
        e_t = perb_p.tile([128, NT], F32, tag="e")
        rs = perb_p.tile([128, 1], F32, tag="rs")
        nc.scalar.activation(
            e_t[:], lg[:], AF.Exp, bias=nmx[:], scale=1.0, accum_out=rs[:]
        )
        rst_ps = ps_tp.tile([1, 128], F32, tag="tp")
        nc.tensor.transpose(rst_ps[:], rs[:], ident[:])
        rsr = perb_p.tile([1, 128], F32, tag="rsr")
        nc.scalar.copy(rsr[:], rst_ps[:])
        tot = perb_p.tile([1, 1], F32, tag="tot")
        nc.vector.tensor_reduce(tot[:], rsr[:], axis=AX.X, op=OP.add)
        inv = perb_p.tile([1, 1], F32, tag="inv")
        nc.vector.reciprocal(inv[:], tot[:])
        inv_ps = ps_tp.tile([128, 1], F32, tag="tp")
        _mm(nc, inv_ps[:], ones_row[:], inv[:], start=True, stop=True)
        invb = perb_p.tile([128, 1], F32, tag="invb")
        nc.scalar.copy(invb[:], inv_ps[:])
        w2 = perb_p.tile([128, NT], F32, tag="w2")
        nc.vector.tensor_tensor(out=w2[:], in0=e_t[:], in1=rstd[:], op=OP.mult)
        nc.vector.tensor_scalar(w2[:], w2[:], invb[:], None, OP.mult)
        # pass 2: xw = sum_t w2*x  (PE bf16 on resident x), S2 = sum_t w2*mu
        w2_bf = perb_p.tile([128, NT], BF16, tag="w2bf")
        nc.vector.tensor_copy(w2_bf[:], w2[:])
        xw_ps = ps_row.tile([1, D], F32, tag="rowp")
        s2_ps = ps_tp.tile([1, 1], F32, tag="tp")
        for ti in range(NT):
            for df in range(2):
                nc.tensor.matmul(
                    xw_ps[:, df * 512 : (df + 1) * 512],
                    w2_bf[:, ti : ti + 1],
                    x_tiles[ti][:, df * 512 : (df + 1) * 512],
                    start=(ti == 0),
                    stop=(ti == NT - 1),
                )
            _mm(
                nc,
                s2_ps[:],
                w2[:, ti : ti + 1],
                stats_b[:, ti, 0:1],
                start=(ti == 0),
                stop=(ti == NT - 1),
            )
        s2_sb = perb_p.tile([1, 1], F32, tag="s2")
        nc.scalar.copy(s2_sb[:], s2_ps[:])
        zg_row = rows_p.tile([1, D], F32, tag="zg")
        nc.vector.tensor_scalar(zg_row[:], xw_ps[:], s2_sb[:], None, OP.subtract)
        nc.vector.tensor_tensor(out=zg_row[:], in0=zg_row[:], in1=g_row[:], op=OP.mult)
        zg_bf = rows_p.tile([1, D], BF16, tag="zgbf")
        nc.scalar.copy(zg_bf[:], zg_row[:])
        zgc_ps = ps_tp.tile([128, ND, 8], BF16, tag="tp")
        for dc in range(ND):
            nc.tensor.transpose(
                zgc_ps[:, dc, 0:1],
                zg_bf[:, dc * 128 : (dc + 1) * 128],
                ident_bf[:1, :1],
            )
        zg_cols = rows_p.tile([128, ND], BF16, tag="zgcols")
        nc.scalar.copy(zg_cols[:], zgc_ps[:, :, 0])
        # out_raw = zg @ Wv.T  (row [1, H])
        out_ps = ps_row.tile([1, H], F32, tag="rowp")
        for dc in range(ND):
            for hf in range(2):
                nc.tensor.matmul(
                    out_ps[:, hf * 512 : (hf + 1) * 512],
                    zg_cols[:, dc : dc + 1],
                    wvT[dc][:, hf * 512 : (hf + 1) * 512],
                    start=(dc == 0),
                    stop=(dc == ND - 1),
                )
        out_bf = rows_p.tile([1, H], BF16, tag="outbf")
        nc.scalar.copy(out_bf[:], out_ps[:])
        oc_ps = ps_tp.tile([128, NH, 8], BF16, tag="tp")
        for hc in range(NH):
            nc.tensor.transpose(
                oc_ps[:, hc, 0:1],
                out_bf[:, hc * 128 : (hc + 1) * 128],
                ident_bf[:1, :1],
            )
        out_cols = rows_p.tile([128, NH], BF16, tag="outcols")
        nc.scalar.copy(out_cols[:], oc_ps[:, :, 0])
        # proj = out @ Wp.T + cvb @ Wp.T  (cpv_row precomputed)
        proj_ps = ps_row.tile([1, D], F32, tag="rowp")
        for hc in range(NH):
            for df in range(2):
                nc.tensor.matmul(
                    proj_ps[:, df * 512 : (df + 1) * 512],
                    out_cols[:, hc : hc + 1],
                    wpT[hc][:, df * 512 : (df + 1) * 512],
                    start=(hc == 0),
                    stop=(hc == NH - 1),
                )
        proj_sb = rows_p.tile([1, D], F32, tag="projsb")
        nc.vector.tensor_tensor(out=proj_sb[:], in0=proj_ps[:], in1=cpv_row[:], op=OP.add)
        # broadcast proj over 128 partitions
        pb_ps = ps_big.tile([128, D], F32, tag="big")
        for df in range(2):
            _mm(
                nc,
                pb_ps[:, df * 512 : (df + 1) * 512],
                ones_row[:],
                proj_sb[:, df * 512 : (df + 1) * 512],
                start=True,
                stop=True,
            )
        proj_bc = pjbc_p.tile([128, 1, D], F32, tag="pjbc")
        nc.scalar.copy(proj_bc[:, 0, :], pb_ps[:])
        # y = x + proj (GPSIMD: bf16 x + f32 proj -> f32) -> DMA out
        for ti in range(NT):
            yt = y_p.tile([128, D], F32, tag="y")
            nc.gpsimd.tensor_tensor(
                out=yt[:], in0=x_tiles[ti][:],
                in1=proj_bc[:, 0, :], op=OP.add,
            )
            nc.sync.dma_start(out_ext[b, ti * 128 : (ti + 1) * 128, :], yt[:])

def build_nc(loop_n: int = 1, split_waits: bool = True):
    nc = bass.Bass("TRN2", target_bir_lowering=False, debug=False)
    ext = {
        "x": nc.declare_dram_parameter("x", [BL, T, D], F32, isOutput=False).ap(),
        "cat_emb": nc.declare_dram_parameter("cat_emb", [BL, C], F32, isOutput=False).ap(),
        "ln_x_g": nc.declare_dram_parameter("ln_x_g", [1, D], F32, isOutput=False).ap(),
        "ln_x_b": nc.declare_dram_parameter("ln_x_b", [1, D], F32, isOutput=False).ap(),
        "ln_c_g": nc.declare_dram_parameter("ln_c_g", [1, C], F32, isOutput=False).ap(),
        "ln_c_b": nc.declare_dram_parameter("ln_c_b", [1, C], F32, isOutput=False).ap(),
        "Wq": nc.declare_dram_parameter("Wq", [H, C], F32, isOutput=False).ap(),
        "Wk": nc.declare_dram_parameter("Wk", [H, D], F32, isOutput=False).ap(),
        "Wv": nc.declare_dram_parameter("Wv", [H, D], F32, isOutput=False).ap(),
        "Wp": nc.declare_dram_parameter("Wp", [D, H], F32, isOutput=False).ap(),
        "out": nc.declare_dram_parameter("out", [BL, T, D], F32, isOutput=True).ap(),
    }
    with PatchedTileContext(nc) as tc:
        with ExitStack() as ctx:
            if loop_n > 1:
                with tc.For_i(0, loop_n, 1):
                    build_body(ctx, tc, ext)
            else:
                build_body(ctx, tc, ext)
    if split_waits:
        split_multi_waits(nc)
    return nc


_NC_CACHE = {}


def _get_nc(loop_n=1):
    if loop_n not in _NC_CACHE:
        _NC_CACHE[loop_n] = build_nc(loop_n)
    return _NC_CACHE[loop_n]


def make_in_maps(inputs):
    x = np.ascontiguousarray(inputs["x"], dtype=np.float32)
    cat = np.ascontiguousarray(inputs["cat_emb"], dtype=np.float32)
    shared = {
        "ln_x_g": np.ascontiguousarray(inputs["ln_x_g"], np.float32).reshape(1, D),
        "ln_x_b": np.ascontiguousarray(inputs["ln_x_b"], np.float32).reshape(1, D),
        "ln_c_g": np.ascontiguousarray(inputs["ln_c_g"], np.float32).reshape(1, C),
        "ln_c_b": np.ascontiguousarray(inputs["ln_c_b"], np.float32).reshape(1, C),
        "Wq": np.ascontiguousarray(inputs["Wq"], np.float32),
        "Wk": np.ascontiguousarray(inputs["Wk"], np.float32),
        "Wv": np.ascontiguousarray(inputs["Wv"], np.float32),
        "Wp": np.ascontiguousarray(inputs["Wp"], np.float32),
    }
    in_maps = []
    for i in range(NCORES):
        m = dict(shared)
        m["x"] = x[i * BL : (i + 1) * BL]
        m["cat_emb"] = cat[i * BL : (i + 1) * BL]
        in_maps.append(m)
    return in_maps


def kernel(**inputs) -> np.ndarray:
    nc = _get_nc(1)
    in_maps = make_in_maps(inputs)
    res = run_bass_kernel_spmd(nc, in_maps, core_ids=list(range(NCORES)))
    out = np.concatenate([res.results[i]["out"] for i in range(NCORES)], axis=0)
    return out.astype(np.float32)

